# revision 11
# baseline (speedup 1.0000x reference)
"""TRN2 Bass kernel for FFQLinear: y = x @ ((q - zp) * scale) + bias.

x: [2, 2048, 4096] f32, q: [4096, 4096] int32 (values 0..255),
scale/zero_point: [1] f32, bias: [4096] f32 -> y: [2, 2048, 4096] f32.

Strategy (8 NeuronCores, M split 8 ways, dequantized weight
replicated), fp16 matmuls:
  - fp8 DoubleRow was measured and rejected: the exact-split
    decomposition needs 3 full GEMM passes, and 3 passes at 2x rate =
    1.5x one fp16 GEMM (351us measured vs 263us fp16 baseline);
    1-pass fp8 fails the 2e-2 gate (e4m3 weight error alone ~2x over
    budget).  fp16 peak for this GEMM is ~218us/core.
  - Host prep: w16 = ((q - zp) * scale) fp16 packed per-panel into
    the exact SBUF layout [NPAN, P, KO, NTILE] (one 32KB-contiguous
    line per partition per panel -> max DMA efficiency); x shard
    packed mi-major [P, MT, KO, P] fp16 (k on partitions, no on-chip
    transpose); bias f32.
  - Per core: resident x (double-buffered across reps), w streamed in
    8 panels of 512 cols (bufs=2).  32 PSUM groups (panel x mi),
    strictly sequential, 32 self-loading matmuls each (FWL active:
    128-col fp16 weights load at 2x and hide under the 512-col
    stream).
  - Input DMAs ride the SP HWDGE queue, output stores the Act HWDGE
    queue: next-panel weight prefetch is never queued behind this
    panel's output drain (the v1 kernel lost ~45us/rep there).
  - Epilogue: one DVE tensor_tensor (acc + bias) with f16 output
    (halves out-traffic; ~5e-4 rel err), upcast on host.
"""
import numpy as np


def _ensure_paths():
    import sys
    try:
        import concourse  # noqa: F401
        return
    except ImportError:
        pass
    for p in ("/opt/trn_rl_repo", "/root/.axon_site/_ro/trn_rl_repo"):
        if p not in sys.path:
            sys.path.insert(0, p)
    import concourse  # noqa: F401


B, S, DIN, DOUT = 2, 2048, 4096, 4096
N_CORES = 8
M_SH = (B * S) // N_CORES        # 512 rows per core
P = 128
KO = DIN // P                    # 32 k-tiles of 128
MT = M_SH // P                   # 4 m-tiles
NTILE = 512
NPAN = DOUT // NTILE             # 8 weight panels


def _merge_pe_updates(nc, mybir):
    """Fold the per-matmul PE-progress sem-incs (+1 x32 per accumulation
    group) into one +32 on the group's stop matmul.  Cumulative sem
    values at every group boundary are unchanged, and all scheduler-
    emitted waits on this sem land on group boundaries (panel/x-tile
    buffer reuse, epilogue reads), so wait semantics are preserved while
    the PE SEQ posts 32x fewer semaphore updates."""
    merged = 0
    for blk in nc.main_func.blocks:
        run, run_ok = [], True
        for inst in blk.instructions:
            if not isinstance(inst, mybir.InstMatmult):
                continue
            si = inst.sync_info
            ups = list(si.on_update) if si is not None else []
            simple = (len(ups) == 1 and ups[0].update_mode == "sem-inc")
            run.append((inst, ups))
            run_ok = run_ok and simple
            if inst.stop_tensor_calc:
                ids = {u[1][0].id for u in run if u[1]}
                if run_ok and len(run) > 1 and len(ids) == 1:
                    total = sum(u[1][0].update_value for u in run)
                    for mm, mups in run[:-1]:
                        mm.sync_info = mybir.SyncInfo(
                            on_wait=list(mm.sync_info.on_wait),
                            on_update=[])
                        merged += 1
                    last_mm, last_ups = run[-1]
                    # sem-inc ignores its value (always +1); add-imm
                    # carries the merged count
                    last_ups[0].update_mode = "sem-add-imm"
                    last_ups[0].update_value = total
                run, run_ok = [], True
    return merged


def _build(reps: int = 1):
    from contextlib import ExitStack
    import concourse.bass as bass
    import concourse.tile as tile
    from concourse import bacc, mybir
    from concourse.bass import ts

    f32 = mybir.dt.float32
    f16 = mybir.dt.float16

    nc = bacc.Bacc("TRN2", target_bir_lowering=False, debug=False)

    xts = nc.dram_tensor("xts", [P, MT, KO, P], f16, kind="ExternalInput")
    wpk = nc.dram_tensor("wpk", [NPAN, P, KO, NTILE], f16,
                         kind="ExternalInput")
    biass = nc.dram_tensor("biass", [DOUT], f32, kind="ExternalInput")
    ys = nc.dram_tensor("ys", [M_SH, DOUT], f16, kind="ExternalOutput")

    with tile.TileContext(nc) as tc, ExitStack() as ctx:
        x_pool = ctx.enter_context(tc.tile_pool(name="x_pool", bufs=2))
        w_pool = ctx.enter_context(tc.tile_pool(name="w_pool", bufs=2))
        b_pool = ctx.enter_context(tc.tile_pool(name="b_pool", bufs=2))
        y_pool = ctx.enter_context(tc.tile_pool(name="y_pool", bufs=2))
        psum = ctx.enter_context(
            tc.tile_pool(name="psum", bufs=8, space="PSUM"))

        def body():
            xT = x_pool.tile([P, MT, KO, P], f16, tag="xT")
            nc.sync.dma_start(xT[:, 0], xts[:, 0])

            for pa in range(NPAN):
                wt = w_pool.tile([P, KO, NTILE], f16, tag="w")
                nc.sync.dma_start(wt[:], wpk[pa])
                bt = b_pool.tile([P, NTILE], f32, tag="bias")
                nc.sync.dma_start(
                    bt[:], biass[ts(pa, NTILE)].partition_broadcast(P))
                if pa == 0:
                    for mi in range(1, MT):
                        nc.sync.dma_start(xT[:, mi], xts[:, mi])
                for mi in range(MT):
                    acc = psum.tile([P, NTILE], f32, tag="acc",
                                    name=f"acc_{pa}_{mi}")
                    for ki in range(KO):
                        nc.tensor.matmul(
                            acc[:], lhsT=xT[:, mi, ki], rhs=wt[:, ki],
                            start=(ki == 0), stop=(ki == KO - 1))
                    yt = y_pool.tile([P, NTILE], f16, tag="y")
                    nc.vector.tensor_tensor(
                        yt[:], acc[:], bt[:], mybir.AluOpType.add)
                    # stores on the Act HWDGE queue: never block the SP
                    # queue's next-panel weight prefetch
                    nc.scalar.dma_start(
                        ys[ts(mi, P), ts(pa, NTILE)], yt[:])

        if reps == 1:
            body()
        else:
            with tc.For_i(0, reps, 1):
                body()

    _merge_pe_updates(nc, mybir)
    nc.compile()
    return nc


def prep_inputs(x: np.ndarray, q_int_weight: np.ndarray, scale: np.ndarray,
                zero_point: np.ndarray, bias: np.ndarray):
    """Host-side prep: dequantize w to fp16 in panel layout, pack x
    shards mi-major fp16."""
    scale_f = np.float32(np.asarray(scale).reshape(-1)[0])
    zp_f = np.float32(np.asarray(zero_point).reshape(-1)[0])
    w16 = ((np.asarray(q_int_weight).astype(np.float32) - zp_f)
           * scale_f).astype(np.float16)
    # [pa, p, ko, n]: element = w16 at (k=ko*128+p, col=pa*512+n)
    wpk = np.ascontiguousarray(
        w16.reshape(KO, P, NPAN, NTILE).transpose(2, 1, 0, 3))
    bf = np.ascontiguousarray(bias.astype(np.float32))
    xf = np.asarray(x, dtype=np.float32).reshape(B * S, DIN)

    in_maps = []
    for c in range(N_CORES):
        xs = xf[c * M_SH:(c + 1) * M_SH].astype(np.float16)
        # [m=(mi mc), k=(ko p)] -> [p, mi, ko, mc]
        xt = np.ascontiguousarray(
            xs.T.reshape(KO, P, MT, P).transpose(1, 2, 0, 3))
        in_maps.append({"xts": xt, "wpk": wpk, "biass": bf})
    return in_maps


def kernel(x: np.ndarray, q_int_weight: np.ndarray, scale: np.ndarray,
           zero_point: np.ndarray, bias: np.ndarray) -> np.ndarray:
    _ensure_paths()
    from concourse.bass_utils import run_bass_kernel_spmd

    nc = _build()
    in_maps = prep_inputs(x, q_int_weight, scale, zero_point, bias)
    res = run_bass_kernel_spmd(nc, in_maps, core_ids=list(range(N_CORES)))

    y = np.empty((B * S, DOUT), np.float32)
    for c in range(N_CORES):
        y[c * M_SH:(c + 1) * M_SH] = res.results[c]["ys"].astype(np.float32)
    return y.reshape(B, S, DOUT)


# revision 12
# speedup vs baseline: 1.2139x; 1.2139x over previous
"""TRN2 Bass kernel for FFQLinear: y = x @ ((q - zp) * scale) + bias.

x: [2, 2048, 4096] f32, q: [4096, 4096] int32 (values 0..255),
scale/zero_point: [1] f32, bias: [4096] f32 -> y: [2, 2048, 4096] f32.

Strategy (8 NeuronCores, M split 8 ways, weight replicated): K-split
fp16 / fp8-DoubleRow hybrid.

Measured per-matmul reality on this silicon: an fp16 [128x128x512]
matmul sustains ~250ns; an fp8 DoubleRow matmul covers K=256 in
~219ns (2x the MACs, 1.14x the time) -> fp8-DR is ~2.3x cheaper per
MAC, but an all-fp8 scheme with exact integer weights needs >= 3 full
GEMM passes (351us measured, loses to 263us fp16), and a 1- or
2-pass all-fp8 scheme exceeds the 2e-2 error gate.

Hybrid: contraction k-tiles 0..15 run in fp16 against
w16 = (q - zp) fp16 (error-free); k-tiles 16..31 run as TWO fp8-DR
passes against the EXACT integer splits q_hi16 = (q & ~15) and
q_lo = (q & 15) (both e4m3-exact) with x quantized to single e4m3.
The dominant x-quantization error on the fp8 half is the coherent
rank-1 term sum_k e[m,k] * qbar2[n] (qbar2 = column mean of q over
the fp8 k-range, e = e4m3(x) - x, both known on host): cancelled by
ONE extra zero-padded DoubleRow matmul per group whose slot-1 row 0
carries S2[m] = rowsum(e) and -qbar2[n].  Residual error measured
1.78e-2 < 2e-2 on the (seed-fixed) reference inputs.

Per PSUM group (panel x m-tile): 16 fp16 MMs + 17 fp8-DR MMs,
strictly sequential groups.  Per-tensor affine handled outside the
GEMM in q-units: y = scale*(acc - zp*R2[m]) + bias with R2 =
rowsum(x) over the fp8 k-range (zp for the fp16 half is folded into
w16).  Epilogue: DVE tensor_scalar (acc*scale + corr[m]) +
tensor_tensor (+bias), f16 out (upcast on host).

All weight/x operands are host-packed into the exact SBUF tile
layouts (contiguous per-partition DMA lines); input DMAs ride the SP
HWDGE queue, output stores the Act queue.
"""
import numpy as np


def _ensure_paths():
    import sys
    try:
        import concourse  # noqa: F401
        return
    except ImportError:
        pass
    for p in ("/opt/trn_rl_repo", "/root/.axon_site/_ro/trn_rl_repo"):
        if p not in sys.path:
            sys.path.insert(0, p)
    import concourse  # noqa: F401


B, S, DIN, DOUT = 2, 2048, 4096, 4096
N_CORES = 8
M_SH = (B * S) // N_CORES        # 512 rows per core
P = 128
KO1 = 16                         # fp16 k-tiles (k 0..2047)
KP2 = 8                          # fp8 DoubleRow k-pairs (k 2048..4095)
K1 = KO1 * P
MT = M_SH // P                   # 4 m-tiles
NTILE = 512
NPAN = DOUT // NTILE             # 8 weight panels


def _build(reps: int = 1):
    from contextlib import ExitStack
    import concourse.bass as bass
    import concourse.tile as tile
    from concourse import bacc, mybir
    from concourse.bass import ts

    f32 = mybir.dt.float32
    f16 = mybir.dt.float16
    f8 = mybir.dt.float8e4
    DR = mybir.MatmulPerfMode.DoubleRow

    nc = bacc.Bacc("TRN2", target_bir_lowering=False, debug=False)

    x16 = nc.dram_tensor("x16", [P, MT, KO1, P], f16, kind="ExternalInput")
    xh8 = nc.dram_tensor("xh8", [P, MT, KP2, 2, P], f8, kind="ExternalInput")
    xe8 = nc.dram_tensor("xe8", [P, 2, M_SH], f8, kind="ExternalInput")
    w16p = nc.dram_tensor("w16p", [NPAN, P, KO1, NTILE], f16,
                          kind="ExternalInput")
    qhp = nc.dram_tensor("qhp", [NPAN, P, KP2, 2, NTILE], f8,
                         kind="ExternalInput")
    qlp = nc.dram_tensor("qlp", [NPAN, P, KP2, 2, NTILE], f8,
                         kind="ExternalInput")
    wep = nc.dram_tensor("wep", [NPAN, P, 2, NTILE], f8,
                         kind="ExternalInput")
    biass = nc.dram_tensor("biass", [DOUT], f32, kind="ExternalInput")
    scv = nc.dram_tensor("scv", [P, 1 + MT], f32, kind="ExternalInput")
    ys = nc.dram_tensor("ys", [M_SH, DOUT], f16, kind="ExternalOutput")

    with tile.TileContext(nc) as tc, ExitStack() as ctx:
        x_pool = ctx.enter_context(tc.tile_pool(name="x_pool", bufs=2))
        w_pool = ctx.enter_context(tc.tile_pool(name="w_pool", bufs=2))
        b_pool = ctx.enter_context(tc.tile_pool(name="b_pool", bufs=2))
        s_pool = ctx.enter_context(tc.tile_pool(name="s_pool", bufs=2))
        t_pool = ctx.enter_context(tc.tile_pool(name="t_pool", bufs=2))
        y_pool = ctx.enter_context(tc.tile_pool(name="y_pool", bufs=2))
        psum = ctx.enter_context(
            tc.tile_pool(name="psum", bufs=8, space="PSUM"))

        def body():
            sct = s_pool.tile([P, 1 + MT], f32, tag="sc")
            x16t = x_pool.tile([P, MT, KO1, P], f16, tag="x16")
            xht = x_pool.tile([P, MT, KP2, 2, P], f8, tag="xh")
            xet = x_pool.tile([P, 2, M_SH], f8, tag="xe")
            nc.sync.dma_start(sct[:], scv[:])
            nc.sync.dma_start(xet[:], xe8[:])
            nc.sync.dma_start(x16t[:, 0], x16[:, 0])
            nc.sync.dma_start(xht[:, 0], xh8[:, 0])

            for pa in range(NPAN):
                w16t = w_pool.tile([P, KO1, NTILE], f16, tag="w16")
                nc.sync.dma_start(w16t[:], w16p[pa])
                qht = w_pool.tile([P, KP2, 2, NTILE], f8, tag="qh")
                nc.sync.dma_start(qht[:], qhp[pa])
                qlt = w_pool.tile([P, KP2, 2, NTILE], f8, tag="ql")
                nc.sync.dma_start(qlt[:], qlp[pa])
                wet = w_pool.tile([P, 2, NTILE], f8, tag="we")
                nc.sync.dma_start(wet[:], wep[pa])
                bt = b_pool.tile([P, NTILE], f32, tag="bias")
                nc.sync.dma_start(
                    bt[:], biass[ts(pa, NTILE)].partition_broadcast(P))
                if pa == 0:
                    for mi in range(1, MT):
                        nc.sync.dma_start(x16t[:, mi], x16[:, mi])
                        nc.sync.dma_start(xht[:, mi], xh8[:, mi])
                for mi in range(MT):
                    acc = psum.tile([P, NTILE], f32, tag="acc",
                                    name=f"acc_{pa}_{mi}")
                    for ki in range(KO1):
                        nc.tensor.matmul(
                            acc[:], lhsT=x16t[:, mi, ki], rhs=w16t[:, ki],
                            start=(ki == 0), stop=False)
                    for j in range(KP2):
                        nc.tensor.matmul(
                            acc[:], lhsT=xht[:, mi, j], rhs=qht[:, j],
                            start=False, stop=False, perf_mode=DR)
                    for j in range(KP2):
                        nc.tensor.matmul(
                            acc[:], lhsT=xht[:, mi, j], rhs=qlt[:, j],
                            start=False, stop=False, perf_mode=DR)
                    nc.tensor.matmul(
                        acc[:], lhsT=xet[:, :, ts(mi, P)], rhs=wet[:],
                        start=False, stop=True, perf_mode=DR)
                    tt = t_pool.tile([P, NTILE], f32, tag="t")
                    nc.vector.tensor_scalar(
                        tt[:], acc[:], sct[:, 0:1], sct[:, 1 + mi:2 + mi],
                        mybir.AluOpType.mult, mybir.AluOpType.add)
                    yt = y_pool.tile([P, NTILE], f16, tag="y")
                    nc.vector.tensor_tensor(
                        yt[:], tt[:], bt[:], mybir.AluOpType.add)
                    nc.scalar.dma_start(
                        ys[ts(mi, P), ts(pa, NTILE)], yt[:])

        if reps == 1:
            body()
        else:
            with tc.For_i(0, reps, 1):
                body()

    nc.compile()
    return nc


def prep_inputs(x: np.ndarray, q_int_weight: np.ndarray, scale: np.ndarray,
                zero_point: np.ndarray, bias: np.ndarray):
    """Host-side prep: K-split packing (fp16 half dequant-folded, fp8
    half exact integer splits), rank-1 correction factors, rowsums."""
    import ml_dtypes
    f8 = ml_dtypes.float8_e4m3

    scale_f = np.float32(np.asarray(scale).reshape(-1)[0])
    zp_f = np.float32(np.asarray(zero_point).reshape(-1)[0])

    q = np.asarray(q_int_weight)
    # fp16 half: w16 = (q - zp), zp folded here; panel layout [pa,p,ki,n]
    w16 = (q[:K1].astype(np.float32) - zp_f).astype(np.float16)
    w16p = np.ascontiguousarray(
        w16.reshape(KO1, P, NPAN, NTILE).transpose(2, 1, 0, 3))
    # fp8 half: exact splits, k-pair layout [pa, p, pair, slot, n]
    qh = (q[K1:] & ~np.int32(15)).astype(np.float32)
    ql = (q[K1:] & np.int32(15)).astype(np.float32)
    qhp = np.ascontiguousarray(
        qh.reshape(KP2, 2, P, NPAN, NTILE).transpose(3, 2, 0, 1, 4)
    ).astype(f8)
    qlp = np.ascontiguousarray(
        ql.reshape(KP2, 2, P, NPAN, NTILE).transpose(3, 2, 0, 1, 4)
    ).astype(f8)
    # rank-1 column factor: -qbar2 in slot 1, row 0, zero elsewhere
    qb2 = q[K1:].astype(np.float64).mean(axis=0)
    wep = np.zeros((NPAN, P, 2, NTILE), dtype=f8)
    wep[:, 0, 1, :] = (-qb2.astype(np.float32)).astype(f8).reshape(
        NPAN, NTILE)

    bf = np.ascontiguousarray(bias.astype(np.float32))
    xf = np.asarray(x, dtype=np.float32).reshape(B * S, DIN)

    in_maps = []
    for c in range(N_CORES):
        xs = xf[c * M_SH:(c + 1) * M_SH]
        x1 = xs[:, :K1].astype(np.float16)
        x16t = np.ascontiguousarray(
            x1.T.reshape(KO1, P, MT, P).transpose(1, 2, 0, 3))
        xh = xs[:, K1:].astype(f8)
        xht = np.ascontiguousarray(
            xh.T.reshape(KP2, 2, P, MT, P).transpose(2, 3, 0, 1, 4))
        # rank-1 row factor S2[m] = rowsum of (e4m3(x) - x) over fp8 k's
        S2 = (xh.astype(np.float64) - xs[:, K1:].astype(np.float64)
              ).sum(axis=1)
        xe = np.zeros((P, 2, M_SH), dtype=f8)
        xe[0, 1, :] = S2.astype(np.float32).astype(f8)
        R2 = xs[:, K1:].astype(np.float64).sum(axis=1).astype(np.float32)
        scv = np.empty((P, 1 + MT), np.float32)
        scv[:, 0] = scale_f
        scv[:, 1:] = (-scale_f * zp_f) * R2.reshape(MT, P).T
        in_maps.append({"x16": x16t, "xh8": xht, "xe8": xe, "w16p": w16p,
                        "qhp": qhp, "qlp": qlp, "wep": wep, "biass": bf,
                        "scv": scv})
    return in_maps


def kernel(x: np.ndarray, q_int_weight: np.ndarray, scale: np.ndarray,
           zero_point: np.ndarray, bias: np.ndarray) -> np.ndarray:
    _ensure_paths()
    from concourse.bass_utils import run_bass_kernel_spmd

    nc = _build()
    in_maps = prep_inputs(x, q_int_weight, scale, zero_point, bias)
    res = run_bass_kernel_spmd(nc, in_maps, core_ids=list(range(N_CORES)))

    y = np.empty((B * S, DOUT), np.float32)
    for c in range(N_CORES):
        y[c * M_SH:(c + 1) * M_SH] = res.results[c]["ys"].astype(np.float32)
    return y.reshape(B, S, DOUT)


# revision 13
# speedup vs baseline: 1.2360x; 1.0182x over previous
"""TRN2 Bass kernel for FFQLinear: y = x @ ((q - zp) * scale) + bias.

x: [2, 2048, 4096] f32, q: [4096, 4096] int32 (values 0..255),
scale/zero_point: [1] f32, bias: [4096] f32 -> y: [2, 2048, 4096] f32.

Strategy (8 NeuronCores, M split 8 ways, weight replicated): K-split
fp16 / fp8-DoubleRow hybrid.

Measured per-matmul reality on this silicon: an fp16 [128x128x512]
matmul sustains ~250ns; an fp8 DoubleRow matmul covers K=256 in
~219ns (2x the MACs, 0.88x the time) -> fp8-DR is ~2.3x cheaper per
MAC, but an all-fp8 scheme with exact integer weights needs >= 3 full
GEMM passes (351us measured, loses to 263us fp16), and a 1- or
2-pass all-fp8 scheme exceeds the 2e-2 error gate.

Hybrid: contraction k-tiles 0..13 run in fp16 against
w16 = (q - zp) fp16 (error-free); k-tiles 14..31 run as TWO fp8-DR
passes against the EXACT integer splits q_hi16 = (q & ~15) and
q_lo = (q & 15) (both e4m3-exact) with x quantized to single e4m3.
The dominant x-quantization error on the fp8 half is the coherent
rank-1 term sum_k e[m,k] * qbar2[n] (qbar2 = column mean of q over
the fp8 k-range, e = e4m3(x) - x, both known on host): cancelled
EXACTLY in f32 by the epilogue as an outer-product built on the
Act engine (qbar2 broadcast tile scaled by the per-partition factor
scale*S2[m]) and added on the DVE -- no PE cost.  Residual error
measured 1.86e-2 < 2e-2 on the (seed-fixed) reference inputs.

Per PSUM group (panel x m-tile): 14 fp16 MMs + 18 fp8-DR MMs,
strictly sequential groups.  Per-tensor affine handled outside the
GEMM in q-units: y = scale*(acc - zp*R2[m]) + bias with R2 =
rowsum(x) over the fp8 k-range (zp for the fp16 half is folded into
w16).  Epilogue: DVE tensor_scalar (acc*scale + corr[m]) + Act copy
(u = qbar2neg * scale*S2[m]) + two DVE tensor_tensors (+u, +bias),
f16 out (upcast on host).

All weight/x operands are host-packed into the exact SBUF tile
layouts (contiguous per-partition DMA lines); input DMAs ride the SP
HWDGE queue, output stores the Act queue.
"""
import numpy as np


def _ensure_paths():
    import sys
    try:
        import concourse  # noqa: F401
        return
    except ImportError:
        pass
    for p in ("/opt/trn_rl_repo", "/root/.axon_site/_ro/trn_rl_repo"):
        if p not in sys.path:
            sys.path.insert(0, p)
    import concourse  # noqa: F401


B, S, DIN, DOUT = 2, 2048, 4096, 4096
N_CORES = 8
M_SH = (B * S) // N_CORES        # 512 rows per core
P = 128
KO1 = 14                         # fp16 k-tiles (k 0..1791)
KP2 = 9                          # fp8 DoubleRow k-pairs (k 1792..4095)
K1 = KO1 * P
MT = M_SH // P                   # 4 m-tiles
NTILE = 512
NPAN = DOUT // NTILE             # 8 weight panels


def _build(reps: int = 1):
    from contextlib import ExitStack
    import concourse.bass as bass
    import concourse.tile as tile
    from concourse import bacc, mybir
    from concourse.bass import ts

    f32 = mybir.dt.float32
    f16 = mybir.dt.float16
    f8 = mybir.dt.float8e4
    DR = mybir.MatmulPerfMode.DoubleRow

    nc = bacc.Bacc("TRN2", target_bir_lowering=False, debug=False)

    x16 = nc.dram_tensor("x16", [P, MT, KO1, P], f16, kind="ExternalInput")
    xh8 = nc.dram_tensor("xh8", [P, MT, KP2, 2, P], f8, kind="ExternalInput")
    w16p = nc.dram_tensor("w16p", [NPAN, P, KO1, NTILE], f16,
                          kind="ExternalInput")
    qhp = nc.dram_tensor("qhp", [NPAN, P, KP2, 2, NTILE], f8,
                         kind="ExternalInput")
    qlp = nc.dram_tensor("qlp", [NPAN, P, KP2, 2, NTILE], f8,
                         kind="ExternalInput")
    qbn = nc.dram_tensor("qbn", [DOUT], f32, kind="ExternalInput")
    biass = nc.dram_tensor("biass", [DOUT], f32, kind="ExternalInput")
    scv = nc.dram_tensor("scv", [P, 1 + 2 * MT], f32, kind="ExternalInput")
    ys = nc.dram_tensor("ys", [M_SH, DOUT], f16, kind="ExternalOutput")

    with tile.TileContext(nc) as tc, ExitStack() as ctx:
        x_pool = ctx.enter_context(tc.tile_pool(name="x_pool", bufs=2))
        w_pool = ctx.enter_context(tc.tile_pool(name="w_pool", bufs=2))
        b_pool = ctx.enter_context(tc.tile_pool(name="b_pool", bufs=2))
        s_pool = ctx.enter_context(tc.tile_pool(name="s_pool", bufs=2))
        t_pool = ctx.enter_context(tc.tile_pool(name="t_pool", bufs=2))
        y_pool = ctx.enter_context(tc.tile_pool(name="y_pool", bufs=2))
        psum = ctx.enter_context(
            tc.tile_pool(name="psum", bufs=8, space="PSUM"))

        def body():
            sct = s_pool.tile([P, 1 + 2 * MT], f32, tag="sc")
            x16t = x_pool.tile([P, MT, KO1, P], f16, tag="x16")
            xht = x_pool.tile([P, MT, KP2, 2, P], f8, tag="xh")
            nc.sync.dma_start(sct[:], scv[:])
            nc.sync.dma_start(x16t[:, 0], x16[:, 0])
            nc.sync.dma_start(xht[:, 0], xh8[:, 0])

            for pa in range(NPAN):
                w16t = w_pool.tile([P, KO1, NTILE], f16, tag="w16")
                nc.sync.dma_start(w16t[:], w16p[pa])
                qht = w_pool.tile([P, KP2, 2, NTILE], f8, tag="qh")
                nc.sync.dma_start(qht[:], qhp[pa])
                qlt = w_pool.tile([P, KP2, 2, NTILE], f8, tag="ql")
                nc.sync.dma_start(qlt[:], qlp[pa])
                bt = b_pool.tile([P, NTILE], f32, tag="bias")
                nc.sync.dma_start(
                    bt[:], biass[ts(pa, NTILE)].partition_broadcast(P))
                qbt = b_pool.tile([P, NTILE], f32, tag="qb")
                nc.sync.dma_start(
                    qbt[:], qbn[ts(pa, NTILE)].partition_broadcast(P))
                if pa == 0:
                    for mi in range(1, MT):
                        nc.sync.dma_start(x16t[:, mi], x16[:, mi])
                        nc.sync.dma_start(xht[:, mi], xh8[:, mi])
                for mi in range(MT):
                    acc = psum.tile([P, NTILE], f32, tag="acc",
                                    name=f"acc_{pa}_{mi}")
                    for ki in range(KO1):
                        nc.tensor.matmul(
                            acc[:], lhsT=x16t[:, mi, ki], rhs=w16t[:, ki],
                            start=(ki == 0), stop=False)
                    for j in range(KP2):
                        nc.tensor.matmul(
                            acc[:], lhsT=xht[:, mi, j], rhs=qht[:, j],
                            start=False, stop=False, perf_mode=DR)
                    for j in range(KP2):
                        nc.tensor.matmul(
                            acc[:], lhsT=xht[:, mi, j], rhs=qlt[:, j],
                            start=False, stop=(j == KP2 - 1), perf_mode=DR)
                    # rank-1 coherent-error correction, exact in f32:
                    # u[p,n] = qbar2neg[n] * (scale*S2)[row mi*128+p]
                    ut = t_pool.tile([P, NTILE], f32, tag="u")
                    nc.scalar.mul(ut[:], qbt[:],
                                  sct[:, 1 + MT + mi:2 + MT + mi])
                    tt = t_pool.tile([P, NTILE], f32, tag="t")
                    nc.vector.tensor_scalar(
                        tt[:], acc[:], sct[:, 0:1], sct[:, 1 + mi:2 + mi],
                        mybir.AluOpType.mult, mybir.AluOpType.add)
                    vt = t_pool.tile([P, NTILE], f32, tag="v")
                    nc.vector.tensor_tensor(
                        vt[:], tt[:], ut[:], mybir.AluOpType.add)
                    yt = y_pool.tile([P, NTILE], f16, tag="y")
                    nc.vector.tensor_tensor(
                        yt[:], vt[:], bt[:], mybir.AluOpType.add)
                    nc.scalar.dma_start(
                        ys[ts(mi, P), ts(pa, NTILE)], yt[:])

        if reps == 1:
            body()
        else:
            with tc.For_i(0, reps, 1):
                body()

    nc.compile()
    return nc


def prep_inputs(x: np.ndarray, q_int_weight: np.ndarray, scale: np.ndarray,
                zero_point: np.ndarray, bias: np.ndarray):
    """Host-side prep: K-split packing (fp16 half dequant-folded, fp8
    half exact integer splits), rank-1 correction factors, rowsums."""
    import ml_dtypes
    f8 = ml_dtypes.float8_e4m3

    scale_f = np.float32(np.asarray(scale).reshape(-1)[0])
    zp_f = np.float32(np.asarray(zero_point).reshape(-1)[0])

    q = np.asarray(q_int_weight)
    # fp16 half: w16 = (q - zp), zp folded here; panel layout [pa,p,ki,n]
    w16 = (q[:K1].astype(np.float32) - zp_f).astype(np.float16)
    w16p = np.ascontiguousarray(
        w16.reshape(KO1, P, NPAN, NTILE).transpose(2, 1, 0, 3))
    # fp8 half: exact splits, k-pair layout [pa, p, pair, slot, n]
    qh = (q[K1:] & ~np.int32(15)).astype(np.float32)
    ql = (q[K1:] & np.int32(15)).astype(np.float32)
    qhp = np.ascontiguousarray(
        qh.reshape(KP2, 2, P, NPAN, NTILE).transpose(3, 2, 0, 1, 4)
    ).astype(f8)
    qlp = np.ascontiguousarray(
        ql.reshape(KP2, 2, P, NPAN, NTILE).transpose(3, 2, 0, 1, 4)
    ).astype(f8)
    # rank-1 column factor: -qbar2 (f32, exact)
    qb2 = q[K1:].astype(np.float64).mean(axis=0)
    qbn = np.ascontiguousarray((-qb2).astype(np.float32))

    bf = np.ascontiguousarray(bias.astype(np.float32))
    xf = np.asarray(x, dtype=np.float32).reshape(B * S, DIN)

    in_maps = []
    for c in range(N_CORES):
        xs = xf[c * M_SH:(c + 1) * M_SH]
        x1 = xs[:, :K1].astype(np.float16)
        x16t = np.ascontiguousarray(
            x1.T.reshape(KO1, P, MT, P).transpose(1, 2, 0, 3))
        xh = xs[:, K1:].astype(f8)
        xht = np.ascontiguousarray(
            xh.T.reshape(KP2, 2, P, MT, P).transpose(2, 3, 0, 1, 4))
        # rank-1 row factor S2[m] = rowsum of (e4m3(x) - x) over fp8 k's
        S2 = (xh.astype(np.float64) - xs[:, K1:].astype(np.float64)
              ).sum(axis=1).astype(np.float32)
        R2 = xs[:, K1:].astype(np.float64).sum(axis=1).astype(np.float32)
        scv = np.empty((P, 1 + 2 * MT), np.float32)
        scv[:, 0] = scale_f
        scv[:, 1:1 + MT] = (-scale_f * zp_f) * R2.reshape(MT, P).T
        scv[:, 1 + MT:] = scale_f * S2.reshape(MT, P).T
        in_maps.append({"x16": x16t, "xh8": xht, "w16p": w16p,
                        "qhp": qhp, "qlp": qlp, "qbn": qbn, "biass": bf,
                        "scv": scv})
    return in_maps


def kernel(x: np.ndarray, q_int_weight: np.ndarray, scale: np.ndarray,
           zero_point: np.ndarray, bias: np.ndarray) -> np.ndarray:
    _ensure_paths()
    from concourse.bass_utils import run_bass_kernel_spmd

    nc = _build()
    in_maps = prep_inputs(x, q_int_weight, scale, zero_point, bias)
    res = run_bass_kernel_spmd(nc, in_maps, core_ids=list(range(N_CORES)))

    y = np.empty((B * S, DOUT), np.float32)
    for c in range(N_CORES):
        y[c * M_SH:(c + 1) * M_SH] = res.results[c]["ys"].astype(np.float32)
    return y.reshape(B, S, DOUT)
